# revision 6
# baseline (speedup 1.0000x reference)
"""Bahdanau decoder step on 8 Trainium2 NeuronCores.

Strategy (tensor parallel, all f32):
- out_W [V, 2H] is the memory-bound bulk (412MB): shard by vocab, 6400
  padded rows/core, streamed and reduced with fused DVE
  tensor_tensor_reduce ops (matvec) against a partition-replicated y.
- Attention: scores reduce to softmax(enc @ w_hi) where
  w_hi = attn_W[:, H:].T @ other (the h-dependent part of the score is
  constant over positions and cancels in softmax). enc/attn_W are
  sharded by hidden column; partial scores are AllReduced.
- GRU: row-sharded (128 output elements/core); h_new slices AllGathered.
- log_softmax: per-core sum(exp(logits)) AllGathered, combined locally.

Collectives: AllReduce(scores[2048]), AllGather(ctx[128]),
AllGather(h_new[128]), AllGather(sumexp[8]) - all tiny/latency-bound.
"""

import numpy as np

import concourse.mybir as mybir
import concourse.tile as tile
from concourse import bacc
from concourse import bass_utils

H = 1024
S = 2048
V = 50257
N_CORES = 8
V8 = 6400           # padded vocab rows per core (128 partitions x 50)
VP = V8 * N_CORES   # 51200
NT = 50             # vocab tiles per core (columns of logits_col)
NCHUNK = 10         # out_W stream chunks per core
ROWS_PER_CHUNK = V8 // 128 // NCHUNK  # 5 rows per partition per chunk

F32 = mybir.dt.float32
ALU = mybir.AluOpType
ACTF = mybir.ActivationFunctionType
AX = mybir.AxisListType

_CACHE = {}


def _build_nc():
    nc = bacc.Bacc("TRN2", target_bir_lowering=False, debug=False,
                   enable_asserts=True, num_devices=N_CORES)

    # ---- external I/O ----
    enc_cols = nc.dram_tensor("enc_cols", [S, 128], F32, kind="ExternalInput").ap()
    w_attn_cols = nc.dram_tensor("w_attn_cols", [H, 128], F32, kind="ExternalInput").ap()
    other_vec = nc.dram_tensor("other_vec", [H], F32, kind="ExternalInput").ap()
    emb_row = nc.dram_tensor("emb_row", [H], F32, kind="ExternalInput").ap()
    h_vec = nc.dram_tensor("h_vec", [H], F32, kind="ExternalInput").ap()
    h_slice = nc.dram_tensor("h_slice", [128], F32, kind="ExternalInput").ap()
    w_ih_s = nc.dram_tensor("w_ih_s", [384, 2 * H], F32, kind="ExternalInput").ap()
    w_hh_s = nc.dram_tensor("w_hh_s", [384, H], F32, kind="ExternalInput").ap()
    gru_b1 = nc.dram_tensor("gru_b1", [384], F32, kind="ExternalInput").ap()
    gru_b2n = nc.dram_tensor("gru_b2n", [128], F32, kind="ExternalInput").ap()
    out_w_s = nc.dram_tensor("out_w_s", [V8, 2 * H], F32, kind="ExternalInput").ap()
    out_b_s = nc.dram_tensor("out_b_s", [V8], F32, kind="ExternalInput").ap()

    logp_s = nc.dram_tensor("logp_s", [V8], F32, kind="ExternalOutput").ap()
    hidden_o = nc.dram_tensor("hidden_o", [H], F32, kind="ExternalOutput").ap()
    attn_o = nc.dram_tensor("attn_o", [S], F32, kind="ExternalOutput").ap()

    RG = [list(range(N_CORES))]

    with tile.TileContext(nc) as tc:
        with (
            tc.tile_pool(name="consts", bufs=1) as consts,
            tc.tile_pool(name="sb", bufs=1) as sb,
            tc.tile_pool(name="stream", bufs=2) as stream,
            tc.tile_pool(name="psum", bufs=1, space="PSUM") as psum,
            tc.tile_pool(name="dram", bufs=1, space="DRAM") as dram,
        ):
            # ---------- constant / early loads ----------
            ones = consts.tile([128, 1], F32)
            nc.vector.memset(ones[:], 1.0)

            # attention operands (critical path -> sync/HWDGE ring)
            enc_sb = sb.tile([128, 16, 128], F32)  # (p, t, h); s = p*16+t
            nc.sync.dma_start(out=enc_sb[:], in_=enc_cols.rearrange("(p t) h -> p t h", t=16))
            attn_sb = sb.tile([128, 8, 128], F32)  # (p, j, h); attn row = j*128+p
            nc.sync.dma_start(out=attn_sb[:], in_=w_attn_cols.rearrange("(j p) h -> p j h", p=128))
            other_sb = sb.tile([128, 8], F32)      # (p, j) = other[j*128+p]
            nc.sync.dma_start(out=other_sb[:], in_=other_vec.rearrange("(j p) -> p j", p=128))

            # GRU weights / biases (bulk -> scalar/ACT HWDGE ring)
            wih_sb = sb.tile([128, 3, 2 * H], F32)  # (p, gate, k); row = g*128+p
            nc.scalar.dma_start(out=wih_sb[:], in_=w_ih_s.rearrange("(g p) k -> p g k", p=128))
            whh_sb = sb.tile([128, 3, H], F32)
            nc.scalar.dma_start(out=whh_sb[:], in_=w_hh_s.rearrange("(g p) k -> p g k", p=128))
            b1_col = sb.tile([128, 3], F32)
            nc.scalar.dma_start(out=b1_col[:], in_=gru_b1.rearrange("(g p) -> p g", p=128))
            b2n_col = sb.tile([128, 1], F32)
            nc.scalar.dma_start(out=b2n_col[:], in_=gru_b2n.rearrange("(p a) -> p a", a=1))
            h_sl = sb.tile([128, 1], F32)
            nc.scalar.dma_start(out=h_sl[:], in_=h_slice.rearrange("(p a) -> p a", a=1))
            out_b_col = sb.tile([128, NT], F32)
            nc.scalar.dma_start(out=out_b_col[:], in_=out_b_s.rearrange("(p t) -> p t", t=NT))

            # partition-replicated vectors
            x_emb_rep = sb.tile([128, H], F32)
            nc.scalar.dma_start(out=x_emb_rep[:],
                                in_=emb_row.rearrange("(a k) -> a k", a=1).to_broadcast((128, H)))
            h_rep = sb.tile([128, H], F32)
            nc.scalar.dma_start(out=h_rep[:],
                                in_=h_vec.rearrange("(a k) -> a k", a=1).to_broadcast((128, H)))

            # collective DRAM buffers
            sc_in = dram.tile([S], F32)
            sc_out = dram.tile([S], F32, addr_space="Shared")
            ctx_in = dram.tile([128], F32)
            ctx_out = dram.tile([H], F32, addr_space="Shared")
            hn_in = dram.tile([128], F32)
            hn_out = dram.tile([H], F32, addr_space="Shared")
            se_in = dram.tile([8], F32)
            se_out = dram.tile([8 * N_CORES], F32, addr_space="Shared")
            whi_d = dram.tile([128], F32)
            rs_d = dram.tile([1], F32)
            lse_d = dram.tile([1], F32)

            # ---------- w_hi = attn_W[:, H+slice].T @ other  (PE) ----------
            w_ps = psum.tile([1, 128], F32)
            for j in range(8):
                nc.tensor.matmul(w_ps[:], lhsT=other_sb[:, j:j + 1], rhs=attn_sb[:, j, :],
                                 start=(j == 0), stop=(j == 7))
            whi_row = sb.tile([1, 128], F32)
            nc.vector.tensor_copy(whi_row[:], w_ps[:])
            nc.sync.dma_start(out=whi_d[:], in_=whi_row[:])
            whi_rep = sb.tile([128, 128], F32)
            nc.sync.dma_start(out=whi_rep[:],
                              in_=whi_d[:].rearrange("(a k) -> a k", a=1).to_broadcast((128, 128)))

            # ---------- partial scores (DVE fused mul+reduce) ----------
            scores_col = sb.tile([128, 16], F32)
            d128 = sb.tile([128, 1], F32)
            for t in range(16):
                nc.vector.scalar_tensor_tensor(
                    out=d128[:].broadcast_to((128, 128)),
                    in0=enc_sb[:, t, :], scalar=1.0, in1=whi_rep[:],
                    op0=ALU.mult, op1=ALU.mult,
                    accum_out=scores_col[:, t:t + 1])
            nc.sync.dma_start(out=sc_in[:].rearrange("(p t) -> p t", t=16), in_=scores_col[:])
            nc.gpsimd.collective_compute("AllReduce", ALU.add, replica_groups=RG,
                                         ins=[sc_in.opt()], outs=[sc_out.opt()])

            # ---------- GRU partial pre-activations that don't need ctx ----------
            gg0 = sb.tile([128, 3], F32)   # W_ih[:, :H] @ emb + b1
            gg2 = sb.tile([128, 3], F32)   # W_hh @ h + (gg0 for r,z ; b_hn for n)
            acc0 = sb.tile([128, 3], F32)
            acc2 = sb.tile([128, 3], F32)
            d1024 = sb.tile([128, 1], F32)
            for g in range(3):
                nc.vector.scalar_tensor_tensor(
                    out=d1024[:].broadcast_to((128, H)),
                    in0=wih_sb[:, g, 0:H], scalar=1.0, in1=x_emb_rep[:],
                    op0=ALU.mult, op1=ALU.mult,
                    accum_out=acc0[:, g:g + 1])
            nc.vector.tensor_add(gg0[:], acc0[:], b1_col[:])
            for g in range(3):
                nc.vector.scalar_tensor_tensor(
                    out=d1024[:].broadcast_to((128, H)),
                    in0=whh_sb[:, g, :], scalar=1.0, in1=h_rep[:],
                    op0=ALU.mult, op1=ALU.mult,
                    accum_out=acc2[:, g:g + 1])
            nc.vector.tensor_add(gg2[:, 0:2], acc2[:, 0:2], gg0[:, 0:2])
            nc.vector.tensor_add(gg2[:, 2:3], acc2[:, 2:3], b2n_col[:])

            # ---------- softmax + context ----------
            ssb = sb.tile([128, 16], F32)
            nc.sync.dma_start(out=ssb[:], in_=sc_out[:].rearrange("(p t) -> p t", t=16))
            esb = sb.tile([128, 16], F32)
            se = sb.tile([128, 1], F32)
            nc.scalar.activation(esb[:], ssb[:], ACTF.Exp, accum_out=se[:])
            st_ps = psum.tile([1, 1], F32)
            nc.tensor.matmul(st_ps[:], lhsT=ones[:], rhs=se[:], start=True, stop=True)
            rs = sb.tile([1, 1], F32)
            nc.vector.reciprocal(rs[:], st_ps[:])

            ctx_ps = psum.tile([1, 128], F32)
            for t in range(16):
                nc.tensor.matmul(ctx_ps[:], lhsT=esb[:, t:t + 1], rhs=enc_sb[:, t, :],
                                 start=(t == 0), stop=(t == 15))
            ctx_row = sb.tile([1, 128], F32)
            nc.vector.tensor_scalar(out=ctx_row[:], in0=ctx_ps[:], scalar1=rs[:],
                                    scalar2=None, op0=ALU.mult)
            nc.sync.dma_start(out=ctx_in[:], in_=ctx_row[:])
            nc.gpsimd.collective_compute("AllGather", ALU.bypass, replica_groups=RG,
                                         ins=[ctx_in.opt()], outs=[ctx_out.opt()])

            # attention weights output (identical on every core)
            nc.sync.dma_start(out=rs_d[:], in_=rs[:])
            rs_rep = sb.tile([128, 1], F32)
            nc.sync.dma_start(out=rs_rep[:],
                              in_=rs_d[:].rearrange("(a k) -> a k", a=1).to_broadcast((128, 1)))
            attn_n = sb.tile([128, 16], F32)
            nc.vector.tensor_scalar(out=attn_n[:], in0=esb[:], scalar1=rs_rep[:],
                                    scalar2=None, op0=ALU.mult)
            nc.scalar.dma_start(out=attn_o[:].rearrange("(p t) -> p t", t=16), in_=attn_n[:])

            # ---------- finish GRU with ctx ----------
            ctx_rep = sb.tile([128, H], F32)
            nc.sync.dma_start(out=ctx_rep[:],
                              in_=ctx_out[:].rearrange("(a k) -> a k", a=1).to_broadcast((128, H)))
            gg3 = sb.tile([128, 3], F32)   # full gate pre-activations (r, z) / gx_n+b_in
            acc3 = sb.tile([128, 3], F32)
            for g in range(3):
                nc.vector.scalar_tensor_tensor(
                    out=d1024[:].broadcast_to((128, H)),
                    in0=wih_sb[:, g, H:2 * H], scalar=1.0, in1=ctx_rep[:],
                    op0=ALU.mult, op1=ALU.mult,
                    accum_out=acc3[:, g:g + 1])
            nc.vector.tensor_add(gg3[:, 0:2], acc3[:, 0:2], gg2[:, 0:2])
            nc.vector.tensor_add(gg3[:, 2:3], acc3[:, 2:3], gg0[:, 2:3])

            # gates via exp (keeps ACT on one table set): sigmoid(a)=1/(1+exp(-a))
            er = sb.tile([128, 2], F32)
            nc.scalar.activation(er[:], gg3[:, 0:2], ACTF.Exp, scale=-1.0)
            er1 = sb.tile([128, 2], F32)
            nc.vector.tensor_scalar_add(er1[:], er[:], 1.0)
            rz = sb.tile([128, 2], F32)
            nc.vector.reciprocal(rz[:], er1[:])
            # n = tanh(gg3_n + r*gg2_n) ; tanh(c) = 2/(1+exp(-2c)) - 1
            cn = sb.tile([128, 1], F32)
            nc.vector.scalar_tensor_tensor(out=cn[:], in0=gg2[:, 2:3], scalar=rz[:, 0:1],
                                           in1=gg3[:, 2:3], op0=ALU.mult, op1=ALU.add)
            ec = sb.tile([128, 1], F32)
            nc.scalar.activation(ec[:], cn[:], ACTF.Exp, scale=-2.0)
            ec1 = sb.tile([128, 1], F32)
            nc.vector.tensor_scalar_add(ec1[:], ec[:], 1.0)
            sg = sb.tile([128, 1], F32)
            nc.vector.reciprocal(sg[:], ec1[:])
            n_t = sb.tile([128, 1], F32)
            nc.vector.tensor_scalar(out=n_t[:], in0=sg[:], scalar1=2.0, scalar2=-1.0,
                                    op0=ALU.mult, op1=ALU.add)
            # h_new = n + z*(h - n)
            t1 = sb.tile([128, 1], F32)
            nc.vector.tensor_sub(t1[:], h_sl[:], n_t[:])
            hn = sb.tile([128, 1], F32)
            nc.vector.scalar_tensor_tensor(out=hn[:], in0=t1[:], scalar=rz[:, 1:2],
                                           in1=n_t[:], op0=ALU.mult, op1=ALU.add)
            nc.sync.dma_start(out=hn_in[:].rearrange("(p a) -> p a", a=1), in_=hn[:])
            nc.gpsimd.collective_compute("AllGather", ALU.bypass, replica_groups=RG,
                                         ins=[hn_in.opt()], outs=[hn_out.opt()])

            # assemble y = [h_new ; ctx], partition-replicated
            y_rep = sb.tile([128, 2 * H], F32)
            nc.sync.dma_start(out=y_rep[:, 0:H],
                              in_=hn_out[:].rearrange("(a k) -> a k", a=1).to_broadcast((128, H)))
            nc.sync.dma_start(out=y_rep[:, H:2 * H],
                              in_=ctx_out[:].rearrange("(a k) -> a k", a=1).to_broadcast((128, H)))
            # hidden output
            hb = sb.tile([1, H], F32)
            nc.scalar.dma_start(out=hb[:], in_=hn_out[:].rearrange("(a k) -> a k", a=1))
            nc.scalar.dma_start(out=hidden_o[:].rearrange("(a k) -> a k", a=1), in_=hb[:])

            # ---------- big matvec: logits = out_W_shard @ y + out_b ----------
            wv = out_w_s.rearrange("(p cj) k -> p cj k", p=128)  # row v = p*50 + cj
            logits_col = sb.tile([128, NT], F32)
            acc_l = sb.tile([128, NT], F32)
            d2048 = sb.tile([128, 1], F32)
            for c in range(NCHUNK):
                wt = stream.tile([128, ROWS_PER_CHUNK, 2 * H], F32)
                nc.gpsimd.dma_start(
                    out=wt[:],
                    in_=wv[:, c * ROWS_PER_CHUNK:(c + 1) * ROWS_PER_CHUNK, :])
                for j in range(ROWS_PER_CHUNK):
                    t = c * ROWS_PER_CHUNK + j
                    nc.vector.scalar_tensor_tensor(
                        out=d2048[:].broadcast_to((128, 2 * H)),
                        in0=wt[:, j, :], scalar=1.0, in1=y_rep[:],
                        op0=ALU.mult, op1=ALU.mult,
                        accum_out=acc_l[:, t:t + 1])
            nc.vector.tensor_add(logits_col[:], acc_l[:], out_b_col[:])

            # ---------- global log-softmax ----------
            el = sb.tile([128, NT], F32)
            se2 = sb.tile([128, 1], F32)
            nc.scalar.activation(el[:], logits_col[:], ACTF.Exp, accum_out=se2[:])
            sl_ps = psum.tile([1, 1], F32)
            nc.tensor.matmul(sl_ps[:], lhsT=ones[:], rhs=se2[:], start=True, stop=True)
            se8 = sb.tile([1, 8], F32)
            nc.vector.memset(se8[:], 0.0)
            nc.vector.tensor_copy(se8[:, 0:1], sl_ps[:])
            nc.sync.dma_start(out=se_in[:].rearrange("(a k) -> a k", a=1), in_=se8[:])
            nc.gpsimd.collective_compute("AllGather", ALU.bypass, replica_groups=RG,
                                         ins=[se_in.opt()], outs=[se_out.opt()])
            sesb = sb.tile([1, 8 * N_CORES], F32)
            nc.sync.dma_start(out=sesb[:], in_=se_out[:].rearrange("(a k) -> a k", a=1))
            st2 = sb.tile([1, 1], F32)
            nc.vector.tensor_reduce(st2[:], sesb[:], axis=AX.X, op=ALU.add)
            lse = sb.tile([1, 1], F32)
            nc.scalar.activation(lse[:], st2[:], ACTF.Ln)
            nc.sync.dma_start(out=lse_d[:], in_=lse[:])
            lse_rep = sb.tile([128, 1], F32)
            nc.sync.dma_start(out=lse_rep[:],
                              in_=lse_d[:].rearrange("(a k) -> a k", a=1).to_broadcast((128, 1)))
            lp = sb.tile([128, NT], F32)
            nc.vector.tensor_scalar(out=lp[:], in0=logits_col[:], scalar1=lse_rep[:],
                                    scalar2=None, op0=ALU.subtract)
            nc.sync.dma_start(out=logp_s[:].rearrange("(p t) -> p t", t=NT), in_=lp[:])

    nc.compile()
    return nc


def _get_nc():
    if "nc" not in _CACHE:
        _CACHE["nc"] = _build_nc()
    return _CACHE["nc"]


def _shard_inputs(last_output_word, last_hidden, encoder_outputs, embedding,
                  attn_W, attn_b, other, W_ih, W_hh, b_ih, b_hh, out_W, out_b):
    f = lambda x: np.asarray(x, dtype=np.float32)
    idx = int(np.asarray(last_output_word).reshape(-1)[0])
    emb = f(np.asarray(embedding)[idx])
    h = f(last_hidden).reshape(-1)[:H]
    enc = f(encoder_outputs).reshape(S, H)
    attn_W = f(attn_W)
    other_v = f(other).reshape(-1)
    W_ih, W_hh = f(W_ih), f(W_hh)
    b_ih, b_hh = f(b_ih).reshape(-1), f(b_hh).reshape(-1)

    W_pad = np.zeros((VP, 2 * H), dtype=np.float32)
    W_pad[:V] = f(out_W)
    b_pad = np.full((VP,), -30000.0, dtype=np.float32)
    b_pad[:V] = f(out_b).reshape(-1)

    in_maps = []
    for c in range(N_CORES):
        sl = slice(c * 128, (c + 1) * 128)
        rows = np.arange(c * 128, (c + 1) * 128)
        gr = np.concatenate([rows, H + rows, 2 * H + rows])
        b1 = np.concatenate([
            b_ih[rows] + b_hh[rows],
            b_ih[H + rows] + b_hh[H + rows],
            b_ih[2 * H + rows],
        ]).astype(np.float32)
        in_maps.append({
            "enc_cols": np.ascontiguousarray(enc[:, sl]),
            "w_attn_cols": np.ascontiguousarray(attn_W[:, H + c * 128:H + (c + 1) * 128]),
            "other_vec": other_v,
            "emb_row": emb,
            "h_vec": h,
            "h_slice": np.ascontiguousarray(h[sl]),
            "w_ih_s": np.ascontiguousarray(W_ih[gr]),
            "w_hh_s": np.ascontiguousarray(W_hh[gr]),
            "gru_b1": b1,
            "gru_b2n": np.ascontiguousarray(b_hh[2 * H + rows]),
            "out_w_s": np.ascontiguousarray(W_pad[c * V8:(c + 1) * V8]),
            "out_b_s": np.ascontiguousarray(b_pad[c * V8:(c + 1) * V8]),
        })
    return in_maps


def run(in_maps, trace=False):
    nc = _get_nc()
    return bass_utils.run_bass_kernel_spmd(
        nc, in_maps, core_ids=list(range(N_CORES)), trace=trace)


def kernel(**inputs):
    in_maps = _shard_inputs(**inputs)
    res = run(in_maps)
    results = res.results
    logp = np.concatenate([results[c]["logp_s"] for c in range(N_CORES)])[:V]
    log_probs = logp[None, :].astype(np.float32)
    hidden = results[0]["hidden_o"][None, None, :].astype(np.float32)
    attn_weights = results[0]["attn_o"][None, None, :].astype(np.float32)
    return (log_probs, hidden, attn_weights)
